# revision 17
# baseline (speedup 1.0000x reference)
"""Causal attention (B=4096, T=64, C=64) on 8 TRN2 NeuronCores, pure data parallel.

Per core: x shard [512, 64, 64]. 512-token macro-tiles (8 batches), bf16 matmul
operands (f32 PSUM accumulate), 2-way tile_position packing: even batches on
out partitions 0:64 (array cols 0:64), odd on 64:128 - only the proven-safe
position set {(0,0), (0,64), (64,64), full-K} is used (mixing array-row
sources into one psum partition range crashes the PE).

Per tile: x DMA'd permuted (partition p <- tokens 4p..4p+3, 1KB descriptors,
one DMA per 2 tiles); Pool casts to bf16; PE transposes bf16 to contiguous
(m p)-packed psum; DVE does a straight 2-byte copy (2x mode) to SBUF; Pool
un-permutes SBUF->SBUF into natural token order (rows 0:64 of a persistent
[65, 512] tile whose row 64 is constant ones).

Folds:
  A = Wq^T Wk: hT = A^T xT in one matmul on the packed xT; the PSUM->SBUF
    cast un-permutes to natural order (Act).
  B_aug65 [65, 65] = [Wv^T Wp^T, 0; bp, 1]: with the ones row of the xT tile
    (K=65), xB = x@B + bp and col 64 = softmax sums source - no bias matmul,
    no ones memset.
  (y + sums*bp) * recip = y*recip + bp: epilogue is one broadcast
    tensor_tensor multiply (DVE) + reciprocal.

Engine split (Pool cannot read PSUM): Pool = cast + un-permute; DVE = xT
copy + recip + scale + part of xB copy; Act = hT copy + exp + rest of xB
copy; y store on HWDGE from SP (SWDGE would cost Pool ~1.2us/tile).
Emission is software-pipelined (12 stages, 1 tile of skew each, reverse-lag
emission order) so no engine's in-order stream waits on the cross-engine
dependency chain; PSUM tags A/B/D/Y ring-buffer across iterations.
"""

import numpy as np
import ml_dtypes

import concourse.bass as bass
import concourse.mybir as mybir
import concourse.tile as tile
import concourse.masks as masks
from concourse import bacc

F32 = mybir.dt.float32
BF16 = mybir.dt.bfloat16

N_CORES = 8
B, T, C = 4096, 64, 64
B_LOC = B // N_CORES  # 512 batches per core

MASK_VAL = -1e9


def build_nc(b_loc=B_LOC, batches_per_tile=8, reps=1, n_stages_dbg=None):
    """Build the single-core Bass graph (SPMD: same graph on all 8 cores)."""
    assert b_loc % batches_per_tile == 0
    n_tiles = b_loc // batches_per_tile
    TOK = batches_per_tile * T              # tokens per macro tile (512)
    NCH = TOK // 128                        # batch pairs per tile (4)
    NB = batches_per_tile                   # batches per tile (8)

    nc = bacc.Bacc("TRN2", target_bir_lowering=False, debug=False)

    x_ext = nc.declare_dram_parameter("x", [b_loc, T, C], F32, isOutput=False)
    Wk_ext = nc.declare_dram_parameter("Wk", [C, C], F32, isOutput=False)
    Wq_ext = nc.declare_dram_parameter("Wq", [C, C], F32, isOutput=False)
    Wv_ext = nc.declare_dram_parameter("Wv", [C, C], F32, isOutput=False)
    Wp_ext = nc.declare_dram_parameter("Wp", [C, C], F32, isOutput=False)
    bp_ext = nc.declare_dram_parameter("bp", [C], F32, isOutput=False)
    out_ext = nc.declare_dram_parameter("out", [b_loc, T, C], F32, isOutput=True)

    x_flat = x_ext.ap().rearrange("b t c -> (b t) c")
    out_flat = out_ext.ap().rearrange("b t c -> (b t) c")

    bf = ml_dtypes.bfloat16
    m1 = np.where(
        np.arange(T)[:, None] <= np.arange(T)[None, :], 0.0, MASK_VAL * 8.0
    ).astype(np.float32)
    maskS2_dram = nc.inline_tensor(
        np.vstack([m1, m1]).astype(bf), name="maskS2_const"
    )
    # one-hot rows for xT_aug (natural order): col t -> row (t % 64)
    identT8 = np.tile(np.eye(T, dtype=np.float32), (1, batches_per_tile))
    identT8_dram = nc.inline_tensor(identT8.astype(bf), name="identT8_const")
    ident_dram = nc.inline_tensor(np.eye(128, dtype=np.float32), name="ident_const")

    AUG_BUFS = 7

    with tile.TileContext(nc) as tc:
        with (
            tc.tile_pool(name="const", bufs=1) as constp,
            tc.tile_pool(name="xin", bufs=4) as xin_pool,
            tc.tile_pool(name="xbf", bufs=3) as xbf_pool,
            tc.tile_pool(name="xp", bufs=3) as xp_pool,
            tc.tile_pool(name="ht", bufs=4) as ht_pool,
            tc.tile_pool(name="wei", bufs=4) as wei_pool,
            tc.tile_pool(name="xb", bufs=3) as xb_pool,
            tc.tile_pool(name="rc", bufs=3) as rc_pool,
            tc.tile_pool(name="yout", bufs=4) as yout_pool,
            tc.tile_pool(name="ps", bufs=2, space="PSUM") as ps,
        ):
            # ---- one-time constants ----
            ident = constp.tile([128, 128], F32)
            nc.sync.dma_start(ident[:], ident_dram.ap())
            ident_bf = constp.tile([128, 128], BF16)
            masks.make_identity(nc, ident_bf[:])
            maskS2 = constp.tile([128, T], BF16)
            nc.sync.dma_start(maskS2[:], maskS2_dram.ap())

            # weights: natural DMA (contiguous), PE transpose, cast to bf16
            wnat = constp.tile([C, 4 * C], F32)
            for i, w_ext in enumerate((Wq_ext, Wk_ext, Wv_ext, Wp_ext)):
                nc.sync.dma_start(wnat[:, i * C : (i + 1) * C], w_ext.ap())
            wT_ps = ps.tile([C, 4 * C], F32, tag="A")
            for i in range(4):
                nc.tensor.transpose(
                    wT_ps[:, i * C : (i + 1) * C],
                    wnat[:, i * C : (i + 1) * C],
                    ident[0:C, 0:C],
                )
            A_ps = ps.tile([C, C], F32, tag="B")
            nc.tensor.matmul(
                A_ps[:], wnat[:, 0 * C : 1 * C], wnat[:, 1 * C : 2 * C]
            )
            A_bf = constp.tile([C, C], BF16)
            nc.vector.tensor_copy(A_bf[:], A_ps[:])
            WpTf = constp.tile([C, C], F32)
            nc.vector.tensor_copy(WpTf[:], wT_ps[:, 3 * C : 4 * C])
            B_ps = ps.tile([C, C], F32, tag="Y")
            nc.tensor.matmul(B_ps[:], wnat[:, 2 * C : 3 * C], WpTf[:])

            # B_aug65 [C+1, C+1]: [Wv^T Wp^T, 0; bp, 1].  With the constant
            # ones row 64 of each xt tile (K=65 matmul), xB = x@B + bp and
            # col 64 = 1 (softmax sums) - no separate bias matmul.
            bp_row = constp.tile([1, C], F32)
            nc.sync.dma_start(bp_row[:], bp_ext.ap().unsqueeze(0))
            B_aug65 = constp.tile([C + 1, C + 1], BF16)
            nc.vector.tensor_copy(B_aug65[0:C, 0:C], B_ps[:])
            nc.vector.memset(B_aug65[0:C, C : C + 1], 0.0)
            nc.vector.tensor_copy(B_aug65[C : C + 1, 0:C], bp_row[:])
            nc.vector.memset(B_aug65[C : C + 1, C : C + 1], 1.0)

            # persistent xT tiles [C+1, TOK]: rows 0:64 rewritten per tile
            # (natural token order), row 64 = constant ones (bias fold)
            XT_BUFS = 6
            xt_tiles = [
                constp.tile(
                    [C + 1, TOK], BF16, tag=f"xt{i}", name=f"xt{i}"
                )
                for i in range(XT_BUFS)
            ]
            for i in range(XT_BUFS):
                nc.vector.memset(xt_tiles[i][C : C + 1, :], 1.0)

            # ---- software-pipelined main loop ----
            # One stage per cross-engine producer/consumer hop, one iteration
            # of skew each, emitted in REVERSE lag order so every consumer
            # precedes next tiles' producers in each engine's program order.
            state = {}

            def s_load(i):
                if i % 2 == 0:
                    st0 = i * TOK
                    x_sb2 = xin_pool.tile([128, 2, NCH * C], F32, tag="x_sb")
                    nc.sync.dma_start(
                        x_sb2[:],
                        x_flat[st0 : st0 + 2 * TOK, :].rearrange(
                            "(u p m) c -> p u (m c)", u=2, m=4
                        ),
                    )
                    state[i] = {"x2": x_sb2}
                    state[i + 1] = {"x2": x_sb2}

            def s_cast(i):
                xbf = xbf_pool.tile([128, NCH * C], BF16, tag="xbf")
                nc.gpsimd.tensor_copy(xbf[:], state[i]["x2"][:, i % 2, :])
                state[i]["xbf"] = xbf

            def s_transpose(i):
                # bf16 transposes to contiguous (m p)-packed psum, then a
                # straight 2-byte DVE copy (2x mode) to SBUF
                xbf = state[i]["xbf"]
                xT_ps = ps.tile([C, TOK], BF16, tag="A")
                for m in range(NCH):
                    nc.tensor.transpose(
                        xT_ps[:, m * 128 : (m + 1) * 128],
                        xbf[:, m * C : (m + 1) * C],
                        ident_bf[:],
                    )
                xp = xp_pool.tile([C, TOK], BF16, tag="xp")
                nc.vector.tensor_copy(xp[:], xT_ps[:])
                state[i]["xp"] = xp

            def s_unperm(i):
                # Pool un-permutes packed -> natural token order (SBUF->SBUF)
                xt = xt_tiles[i % XT_BUFS]
                nc.gpsimd.tensor_copy(
                    xt[0:C, :].rearrange("c (p m) -> c m p", m=4),
                    state[i]["xp"][:].rearrange("c (m p) -> c m p", p=128),
                )
                state[i]["xT"] = xt

            def s_ht(i):
                # hT = A^T x^T on the packed xT (no need to wait for unperm)
                hT_ps = ps.tile([C, TOK], F32, tag="B")
                nc.tensor.matmul(hT_ps[:], A_bf[:], state[i]["xp"][:])
                state[i]["hT_ps"] = hT_ps

            def s_ht_copy(i):
                # un-permute into natural order during the PSUM->SBUF cast
                hT = ht_pool.tile([C, TOK], BF16, tag="hT")
                nc.scalar.copy(
                    hT[:].rearrange("c (p m) -> c m p", m=4),
                    state[i].pop("hT_ps")[:].rearrange("c (m p) -> c m p", p=128),
                )
                state[i]["hT"] = hT

            def s_scores(i):
                xt = state[i]["xT"]
                hT = state[i]["hT"]
                weiT_ps = ps.tile([128, NCH, T], F32, tag="D")
                nc.tensor.matmul(
                    weiT_ps[:],
                    ident_bf[:],
                    maskS2[:].unsqueeze(1).broadcast_to([128, NCH, T]),
                    start=True, stop=False, skip_group_check=True,
                )
                for b in range(NB):
                    j, par = b // 2, b % 2
                    nc.tensor.matmul(
                        weiT_ps[par * T : (par + 1) * T, j, :],
                        xt[0:C, b * T : (b + 1) * T],
                        hT[:, b * T : (b + 1) * T],
                        start=False, stop=(b == NB - 1),
                        tile_position=(0, 64 * par),
                        skip_group_check=True,
                    )
                state[i]["weiT_ps"] = weiT_ps

            def s_exp(i):
                weiT_e = wei_pool.tile([128, NCH, T], BF16, tag="weiT_e")
                nc.scalar.activation(
                    weiT_e[:], state[i].pop("weiT_ps")[:],
                    mybir.ActivationFunctionType.Exp, scale=0.125,
                )
                state[i]["wei"] = weiT_e

            def s_xb(i):
                xt = state[i]["xT"]
                xB_ps = ps.tile([128, NCH, C + 1], F32, tag="A")
                for j in range(NCH):
                    nc.tensor.matmul(
                        xB_ps[:, j, :],
                        xt[:, j * 128 : (j + 1) * 128],
                        B_aug65[:],
                    )
                state[i]["xB_ps"] = xB_ps

            def s_xb_copy(i):
                # split between Act and DVE to balance engine load
                xB_ps = state[i].pop("xB_ps")
                xB = xb_pool.tile([128, NCH, C + 1], BF16, tag="xB")
                nc.scalar.copy(xB[:, :, 0:26], xB_ps[:, :, 0:26])
                nc.vector.tensor_copy(xB[:, :, 26:65], xB_ps[:, :, 26:65])
                state[i]["xB"] = xB

            def s_y(i):
                weiT_e = state[i]["wei"]
                xB = state[i]["xB"]
                y_ps = ps.tile([128, NCH, C + 2], F32, tag="Y")
                for j in range(NCH):
                    nc.tensor.matmul(
                        y_ps[0:T, j, 0 : C + 1],
                        weiT_e[0:T, j, :], xB[0:T, j, :],
                    )
                    nc.tensor.matmul(
                        y_ps[T:128, j, 0 : C + 1],
                        weiT_e[T:128, j, :], xB[T:128, j, :],
                        tile_position=(64, 64),
                    )
                state[i]["y_ps"] = y_ps

            def s_fin(i):
                y_ps = state[i].pop("y_ps")
                recip = rc_pool.tile([128, NCH], F32, tag="recip")
                nc.vector.reciprocal(recip[:], y_ps[:, :, C : C + 1])
                if i % 2 == 0:
                    y2 = yout_pool.tile([128, 2, NCH, C], F32, tag="y_sb")
                    state[i]["y2"] = y2
                    if i + 1 in state:
                        state[i + 1]["y2"] = y2
                y_sb2 = state[i]["y2"]
                nc.vector.tensor_tensor(
                    y_sb2[:, i % 2],
                    y_ps[:, :, 0:C],
                    recip[:].unsqueeze(2).broadcast_to([128, NCH, C]),
                    mybir.AluOpType.mult,
                )
                t0 = i * TOK
                nc.sync.dma_start(
                    out_flat[t0 : t0 + TOK, :].rearrange(
                        "(j p) c -> p j c", p=128
                    ),
                    y_sb2[:, i % 2],
                )
                del state[i]

            stages = [
                (0, s_load),
                (2, s_cast),
                (3, s_transpose),
                (4, s_unperm),
                (4, s_ht),
                (5, s_ht_copy),
                (6, s_scores),
                (7, s_exp),
                (8, s_xb),
                (9, s_xb_copy),
                (10, s_y),
                (11, s_fin),
            ]
            if n_stages_dbg is not None:
                stages = stages[:n_stages_dbg]
            max_lag = stages[-1][0]
            emit_order = sorted(stages, key=lambda s: -s[0])

            rep_ctx = tc.For_i(0, reps, 1) if reps > 1 else None
            if rep_ctx is not None:
                rep_ctx.__enter__()
            for it in range(n_tiles + max_lag):
                for lag, stage in emit_order:
                    i = it - lag
                    if 0 <= i < n_tiles:
                        stage(i)
            if rep_ctx is not None:
                rep_ctx.__exit__(None, None, None)

    nc.compile()
    return nc


_NC_CACHE = {}


def _get_nc(b_loc, batches_per_tile=8):
    key = (b_loc, batches_per_tile)
    if key not in _NC_CACHE:
        _NC_CACHE[key] = build_nc(b_loc, batches_per_tile)
    return _NC_CACHE[key]


def kernel(x, Wk, Wq, Wv, Wp, bp):
    from concourse.bass_utils import run_bass_kernel_spmd

    x = np.ascontiguousarray(x, dtype=np.float32)
    weights = {
        "Wk": np.ascontiguousarray(Wk, dtype=np.float32),
        "Wq": np.ascontiguousarray(Wq, dtype=np.float32),
        "Wv": np.ascontiguousarray(Wv, dtype=np.float32),
        "Wp": np.ascontiguousarray(Wp, dtype=np.float32),
        "bp": np.ascontiguousarray(bp, dtype=np.float32),
    }
    nc = _get_nc(B_LOC)
    in_maps = [
        {"x": x[i * B_LOC : (i + 1) * B_LOC], **weights} for i in range(N_CORES)
    ]
    res = run_bass_kernel_spmd(nc, in_maps, core_ids=list(range(N_CORES)))
    outs = [res.results[i]["out"] for i in range(N_CORES)]
    return np.concatenate(outs, axis=0)


# revision 20
# speedup vs baseline: 1.0212x; 1.0212x over previous
"""Causal attention (B=4096, T=64, C=64) on 8 TRN2 NeuronCores, pure data parallel.

Per core: x shard [512, 64, 64]. 512-token macro-tiles (8 batches), bf16 matmul
operands (f32 PSUM accumulate), 2-way tile_position packing: even batches on
out partitions 0:64 (array cols 0:64), odd on 64:128 - only the proven-safe
position set {(0,0), (0,64), (64,64), full-K} is used (mixing array-row
sources into one psum partition range crashes the PE).

Per tile: x DMA'd permuted (partition p <- tokens 4p..4p+3, 1KB descriptors,
one DMA per 2 tiles); Pool casts to bf16; PE transposes bf16 to contiguous
(m p)-packed psum; DVE does a straight 2-byte copy (2x mode) to SBUF; Pool
un-permutes SBUF->SBUF into natural token order (rows 0:64 of a persistent
[65, 512] tile whose row 64 is constant ones).

Folds:
  A = Wq^T Wk: hT = A^T xT in one matmul on the packed xT; the PSUM->SBUF
    cast un-permutes to natural order (Act).
  B_aug65 [65, 65] = [Wv^T Wp^T, 0; bp, 1]: with the ones row of the xT tile
    (K=65), xB = x@B + bp and col 64 = softmax sums source - no bias matmul,
    no ones memset.
  (y + sums*bp) * recip = y*recip + bp: epilogue is one broadcast
    tensor_tensor multiply (DVE) + reciprocal.

Engine split (Pool cannot read PSUM): Pool = cast + un-permute; DVE = xT
copy + recip + scale + part of xB copy; Act = hT copy + exp + rest of xB
copy; y store on HWDGE from SP (SWDGE would cost Pool ~1.2us/tile).
Emission is software-pipelined (12 stages, 1 tile of skew each, reverse-lag
emission order) so no engine's in-order stream waits on the cross-engine
dependency chain; PSUM tags A/B/D/Y ring-buffer across iterations.
"""

import numpy as np
import ml_dtypes

import concourse.bass as bass
import concourse.mybir as mybir
import concourse.tile as tile
import concourse.masks as masks
from concourse import bacc

F32 = mybir.dt.float32
BF16 = mybir.dt.bfloat16

N_CORES = 8
B, T, C = 4096, 64, 64
B_LOC = B // N_CORES  # 512 batches per core

MASK_VAL = -1e9


def build_nc(b_loc=B_LOC, batches_per_tile=8, reps=1, n_stages_dbg=None):
    """Build the single-core Bass graph (SPMD: same graph on all 8 cores)."""
    assert b_loc % batches_per_tile == 0
    n_tiles = b_loc // batches_per_tile
    TOK = batches_per_tile * T              # tokens per macro tile (512)
    NCH = TOK // 128                        # batch pairs per tile (4)
    NB = batches_per_tile                   # batches per tile (8)

    nc = bacc.Bacc("TRN2", target_bir_lowering=False, debug=False)

    x_ext = nc.declare_dram_parameter("x", [b_loc, T, C], F32, isOutput=False)
    Wk_ext = nc.declare_dram_parameter("Wk", [C, C], F32, isOutput=False)
    Wq_ext = nc.declare_dram_parameter("Wq", [C, C], F32, isOutput=False)
    Wv_ext = nc.declare_dram_parameter("Wv", [C, C], F32, isOutput=False)
    Wp_ext = nc.declare_dram_parameter("Wp", [C, C], F32, isOutput=False)
    bp_ext = nc.declare_dram_parameter("bp", [C], F32, isOutput=False)
    out_ext = nc.declare_dram_parameter("out", [b_loc, T, C], F32, isOutput=True)

    x_flat = x_ext.ap().rearrange("b t c -> (b t) c")
    out_flat = out_ext.ap().rearrange("b t c -> (b t) c")

    bf = ml_dtypes.bfloat16
    m1 = np.where(
        np.arange(T)[:, None] <= np.arange(T)[None, :], 0.0, MASK_VAL * 8.0
    ).astype(np.float32)
    maskS2_dram = nc.inline_tensor(
        np.vstack([m1, m1]).astype(bf), name="maskS2_const"
    )
    # one-hot rows for xT_aug (natural order): col t -> row (t % 64)
    identT8 = np.tile(np.eye(T, dtype=np.float32), (1, batches_per_tile))
    identT8_dram = nc.inline_tensor(identT8.astype(bf), name="identT8_const")
    ident_dram = nc.inline_tensor(np.eye(128, dtype=np.float32), name="ident_const")

    AUG_BUFS = 7

    with tile.TileContext(nc) as tc:
        with (
            tc.tile_pool(name="const", bufs=1) as constp,
            tc.tile_pool(name="xin", bufs=4) as xin_pool,
            tc.tile_pool(name="xp", bufs=3) as xp_pool,
            tc.tile_pool(name="hp", bufs=3) as hp_pool,
            tc.tile_pool(name="ht", bufs=4) as ht_pool,
            tc.tile_pool(name="wei", bufs=4) as wei_pool,
            tc.tile_pool(name="xb", bufs=3) as xb_pool,
            tc.tile_pool(name="rc", bufs=3) as rc_pool,
            tc.tile_pool(name="yout", bufs=4) as yout_pool,
            tc.tile_pool(name="ps", bufs=2, space="PSUM") as ps,
        ):
            # ---- one-time constants ----
            ident = constp.tile([128, 128], F32)
            nc.sync.dma_start(ident[:], ident_dram.ap())
            ident_bf = constp.tile([128, 128], BF16)
            masks.make_identity(nc, ident_bf[:])
            maskS2 = constp.tile([128, T], BF16)
            nc.sync.dma_start(maskS2[:], maskS2_dram.ap())

            # weights: natural DMA (contiguous), PE transpose, cast to bf16
            wnat = constp.tile([C, 4 * C], F32)
            for i, w_ext in enumerate((Wq_ext, Wk_ext, Wv_ext, Wp_ext)):
                nc.sync.dma_start(wnat[:, i * C : (i + 1) * C], w_ext.ap())
            wT_ps = ps.tile([C, 4 * C], F32, tag="A")
            for i in range(4):
                nc.tensor.transpose(
                    wT_ps[:, i * C : (i + 1) * C],
                    wnat[:, i * C : (i + 1) * C],
                    ident[0:C, 0:C],
                )
            A_ps = ps.tile([C, C], F32, tag="B")
            nc.tensor.matmul(
                A_ps[:], wnat[:, 0 * C : 1 * C], wnat[:, 1 * C : 2 * C]
            )
            A_bf = constp.tile([C, C], BF16)
            nc.vector.tensor_copy(A_bf[:], A_ps[:])
            WpTf = constp.tile([C, C], F32)
            nc.vector.tensor_copy(WpTf[:], wT_ps[:, 3 * C : 4 * C])
            B_ps = ps.tile([C, C], F32, tag="Y")
            nc.tensor.matmul(B_ps[:], wnat[:, 2 * C : 3 * C], WpTf[:])

            # B_aug65 [C+1, C+1]: [Wv^T Wp^T, 0; bp, 1].  With the constant
            # ones row 64 of each xt tile (K=65 matmul), xB = x@B + bp and
            # col 64 = 1 (softmax sums) - no separate bias matmul.
            bp_row = constp.tile([1, C], F32)
            nc.sync.dma_start(bp_row[:], bp_ext.ap().unsqueeze(0))
            B_aug65 = constp.tile([C + 1, C + 1], BF16)
            nc.vector.tensor_copy(B_aug65[0:C, 0:C], B_ps[:])
            nc.vector.memset(B_aug65[0:C, C : C + 1], 0.0)
            nc.vector.tensor_copy(B_aug65[C : C + 1, 0:C], bp_row[:])
            nc.vector.memset(B_aug65[C : C + 1, C : C + 1], 1.0)

            # persistent xT tiles [C+1, TOK]: rows 0:64 rewritten per tile
            # (natural token order), row 64 = constant ones (bias fold)
            XT_BUFS = 6
            xt_tiles = [
                constp.tile(
                    [C + 1, TOK], BF16, tag=f"xt{i}", name=f"xt{i}"
                )
                for i in range(XT_BUFS)
            ]
            for i in range(XT_BUFS):
                nc.vector.memset(xt_tiles[i][C : C + 1, :], 1.0)

            # ---- software-pipelined main loop ----
            # One stage per cross-engine producer/consumer hop, one iteration
            # of skew each, emitted in REVERSE lag order so every consumer
            # precedes next tiles' producers in each engine's program order.
            state = {}

            def s_load(i):
                if i % 2 == 0:
                    st0 = i * TOK
                    x_sb2 = xin_pool.tile([128, 2, NCH * C], F32, tag="x_sb")
                    nc.sync.dma_start(
                        x_sb2[:],
                        x_flat[st0 : st0 + 2 * TOK, :].rearrange(
                            "(u p m) c -> p u (m c)", u=2, m=4
                        ),
                    )
                    state[i] = {"x2": x_sb2}
                    state[i + 1] = {"x2": x_sb2}

            def s_transpose(i):
                # f32 transposes to contiguous (m p)-packed psum (partition 0
                # only - a hardware rule for transposes), casting copy after
                x_sb = state[i]["x2"][:, i % 2, :]
                xT_ps = ps.tile([C, TOK], F32, tag="A")
                for m in range(NCH):
                    nc.tensor.transpose(
                        xT_ps[:, m * 128 : (m + 1) * 128],
                        x_sb[:, m * C : (m + 1) * C],
                        ident[:],
                    )
                state[i]["xT_ps"] = xT_ps

            def s_xt_cast(i):
                # straight f32->bf16 cast-copy on Act (measured faster than
                # DVE for plain casts: 614 vs 876 ns)
                xp = xp_pool.tile([C, TOK], BF16, tag="xp")
                nc.scalar.copy(xp[:], state[i].pop("xT_ps")[:])
                state[i]["xp"] = xp

            def s_unperm(i):
                # Pool un-permutes packed -> natural token order (SBUF->SBUF)
                xt = xt_tiles[i % XT_BUFS]
                nc.gpsimd.tensor_copy(
                    xt[0:C, :].rearrange("c (p m) -> c m p", m=4),
                    state[i]["xp"][:].rearrange("c (m p) -> c m p", p=128),
                )
                state[i]["xT"] = xt

            def s_ht(i):
                # hT = A^T x^T on the packed xT (no need to wait for unperm)
                hT_ps = ps.tile([C, TOK], F32, tag="B")
                nc.tensor.matmul(hT_ps[:], A_bf[:], state[i]["xp"][:])
                state[i]["hT_ps"] = hT_ps

            def s_ht_copy(i):
                # straight PSUM->SBUF cast on DVE (packed order)
                hp = hp_pool.tile([C, TOK], BF16, tag="hp")
                nc.vector.tensor_copy(hp[:], state[i].pop("hT_ps")[:])
                state[i]["hp"] = hp

            def s_ht_unperm(i):
                # Pool un-permutes packed -> natural (SBUF->SBUF, cheap on HW)
                hT = ht_pool.tile([C, TOK], BF16, tag="hT")
                nc.gpsimd.tensor_copy(
                    hT[:].rearrange("c (p m) -> c m p", m=4),
                    state[i].pop("hp")[:].rearrange("c (m p) -> c m p", p=128),
                )
                state[i]["hT"] = hT

            def s_scores(i):
                xt = state[i]["xT"]
                hT = state[i]["hT"]
                weiT_ps = ps.tile([128, NCH, T], F32, tag="D")
                nc.tensor.matmul(
                    weiT_ps[:],
                    ident_bf[:],
                    maskS2[:].unsqueeze(1).broadcast_to([128, NCH, T]),
                    start=True, stop=False, skip_group_check=True,
                )
                for b in range(NB):
                    j, par = b // 2, b % 2
                    nc.tensor.matmul(
                        weiT_ps[par * T : (par + 1) * T, j, :],
                        xt[0:C, b * T : (b + 1) * T],
                        hT[:, b * T : (b + 1) * T],
                        start=False, stop=(b == NB - 1),
                        tile_position=(0, 64 * par),
                        skip_group_check=True,
                    )
                state[i]["weiT_ps"] = weiT_ps

            def s_exp(i):
                weiT_e = wei_pool.tile([128, NCH, T], BF16, tag="weiT_e")
                nc.scalar.activation(
                    weiT_e[:], state[i].pop("weiT_ps")[:],
                    mybir.ActivationFunctionType.Exp, scale=0.125,
                )
                state[i]["wei"] = weiT_e

            def s_xb(i):
                xt = state[i]["xT"]
                xB_ps = ps.tile([128, NCH, C + 1], F32, tag="A")
                for j in range(NCH):
                    nc.tensor.matmul(
                        xB_ps[:, j, :],
                        xt[:, j * 128 : (j + 1) * 128],
                        B_aug65[:],
                    )
                state[i]["xB_ps"] = xB_ps

            def s_xb_copy(i):
                xB = xb_pool.tile([128, NCH, C + 1], BF16, tag="xB")
                nc.vector.tensor_copy(xB[:], state[i].pop("xB_ps")[:])
                state[i]["xB"] = xB

            def s_y(i):
                weiT_e = state[i]["wei"]
                xB = state[i]["xB"]
                y_ps = ps.tile([128, NCH, C + 2], F32, tag="Y")
                for j in range(NCH):
                    nc.tensor.matmul(
                        y_ps[0:T, j, 0 : C + 1],
                        weiT_e[0:T, j, :], xB[0:T, j, :],
                    )
                    nc.tensor.matmul(
                        y_ps[T:128, j, 0 : C + 1],
                        weiT_e[T:128, j, :], xB[T:128, j, :],
                        tile_position=(64, 64),
                    )
                state[i]["y_ps"] = y_ps

            def s_fin(i):
                y_ps = state[i].pop("y_ps")
                recip = rc_pool.tile([128, NCH], F32, tag="recip")
                nc.vector.reciprocal(recip[:], y_ps[:, :, C : C + 1])
                if i % 2 == 0:
                    y2 = yout_pool.tile([128, 2, NCH, C], F32, tag="y_sb")
                    state[i]["y2"] = y2
                    if i + 1 in state:
                        state[i + 1]["y2"] = y2
                y_sb2 = state[i]["y2"]
                nc.vector.tensor_tensor(
                    y_sb2[:, i % 2],
                    y_ps[:, :, 0:C],
                    recip[:].unsqueeze(2).broadcast_to([128, NCH, C]),
                    mybir.AluOpType.mult,
                )
                t0 = i * TOK
                nc.sync.dma_start(
                    out_flat[t0 : t0 + TOK, :].rearrange(
                        "(j p) c -> p j c", p=128
                    ),
                    y_sb2[:, i % 2],
                )
                del state[i]

            stages = [
                (0, s_load),
                (2, s_transpose),
                (3, s_xt_cast),
                (4, s_unperm),
                (4, s_ht),
                (5, s_ht_copy),
                (6, s_ht_unperm),
                (7, s_scores),
                (8, s_exp),
                (9, s_xb),
                (10, s_xb_copy),
                (11, s_y),
                (12, s_fin),
            ]
            if n_stages_dbg is not None:
                stages = stages[:n_stages_dbg]
            max_lag = stages[-1][0]
            emit_order = sorted(stages, key=lambda s: -s[0])

            rep_ctx = tc.For_i(0, reps, 1) if reps > 1 else None
            if rep_ctx is not None:
                rep_ctx.__enter__()
            for it in range(n_tiles + max_lag):
                for lag, stage in emit_order:
                    i = it - lag
                    if 0 <= i < n_tiles:
                        stage(i)
            if rep_ctx is not None:
                rep_ctx.__exit__(None, None, None)

    nc.compile()
    return nc


_NC_CACHE = {}


def _get_nc(b_loc, batches_per_tile=8):
    key = (b_loc, batches_per_tile)
    if key not in _NC_CACHE:
        _NC_CACHE[key] = build_nc(b_loc, batches_per_tile)
    return _NC_CACHE[key]


def kernel(x, Wk, Wq, Wv, Wp, bp):
    from concourse.bass_utils import run_bass_kernel_spmd

    x = np.ascontiguousarray(x, dtype=np.float32)
    weights = {
        "Wk": np.ascontiguousarray(Wk, dtype=np.float32),
        "Wq": np.ascontiguousarray(Wq, dtype=np.float32),
        "Wv": np.ascontiguousarray(Wv, dtype=np.float32),
        "Wp": np.ascontiguousarray(Wp, dtype=np.float32),
        "bp": np.ascontiguousarray(bp, dtype=np.float32),
    }
    nc = _get_nc(B_LOC)
    in_maps = [
        {"x": x[i * B_LOC : (i + 1) * B_LOC], **weights} for i in range(N_CORES)
    ]
    res = run_bass_kernel_spmd(nc, in_maps, core_ids=list(range(N_CORES)))
    outs = [res.results[i]["out"] for i in range(N_CORES)]
    return np.concatenate(outs, axis=0)
